# revision 18
# baseline (speedup 1.0000x reference)
"""ANFIS layer kernel for 8 TRN2 NeuronCores.

Math: the rule-strength tensor w[b, r] (B x 65536) is a Kronecker product of
8 per-input membership vectors (4 each).  Split rules r = p*256 + q with
p = digits of inputs 0-3, q = digits of inputs 4-7:
    w[b, p*256+q] = wa[b,p] * wb[b,q]
so with V9[p, q, j] = [consequent_weights | consequent_bias][p*256+q, j]:
    num[b]  = sum_j x9[b,j] * (wa[b]^T V9[:,:,j] wb[b])      (x9 = [x, 1])
    wsum[b] = sum(wa[b]) * sum(wb[b])
    out[b]  = num[b] / (wsum[b] + 1e-6)
The 65536-wide tensor is never materialized; per core the heavy op is a
(128 x 256) @ (256 x 2304) matmul.  Batch is sharded across the 8 cores.

Sync discipline: EVERY read-after-write needs a semaphore edge, including
same-engine consecutive ops (verified on HW: unsynced same-engine RAW reads
stale data), and outputs must not alias inputs.
"""

import numpy as np

import concourse.bass as bass
import concourse.mybir as mybir
from concourse.bass_utils import run_bass_kernel_spmd

N_CORES = 8
B = 1024
BL = B // N_CORES  # 128 batch rows per core
NI = 8             # inputs
NMF = 4            # membership functions per input
P = NMF ** 4       # 256
NJ = NI + 1        # 8 weight columns + bias
NW = NJ * P        # 2304 output columns of the matmul

F32 = mybir.dt.float32
BF16 = mybir.dt.bfloat16


def build_nc(debug: bool = False):
    nc = bass.Bass(trn_type="TRN2")

    # register 1e-5 as a const AP (used as activation bias)
    eps_t = nc.alloc_sbuf_tensor("const-eps5", [128, 1], F32)
    nc.gpsimd.memset(eps_t.ap(), 1e-5)
    nc.const_aps.aps[(F32, 1e-5)] = eps_t.ap()
    nc.all_engine_barrier()

    x_e = nc.declare_dram_parameter("x", [BL, NI], F32, isOutput=False)
    musig_e = nc.declare_dram_parameter("musig", [2, 32], F32, isOutput=False)
    w_e = nc.declare_dram_parameter("wmat", [2, 128, NW], BF16, isOutput=False)
    id_e = nc.declare_dram_parameter("ident", [128, 128], F32, isOutput=False)
    out_e = nc.declare_dram_parameter("out", [BL, 1], F32, isOutput=True)

    # SBUF
    x_sb = nc.alloc_sbuf_tensor("x_sb", [BL, NI], F32)
    id_sb = nc.alloc_sbuf_tensor("id_sb", [128, 128], F32)
    mu_b = nc.alloc_sbuf_tensor("mu_b", [BL, 32], F32)
    sig_b = nc.alloc_sbuf_tensor("sig_b", [BL, 32], F32)
    sabs = nc.alloc_sbuf_tensor("sabs", [BL, 32], F32)
    v2_b = nc.alloc_sbuf_tensor("v2_b", [BL, 32], F32)
    rc_b = nc.alloc_sbuf_tensor("rc_b", [BL, 32], F32)
    d_b = nc.alloc_sbuf_tensor("d_b", [BL, 32], F32)
    dsq_b = nc.alloc_sbuf_tensor("dsq_b", [BL, 32], F32)
    e_b = nc.alloc_sbuf_tensor("e_b", [BL, 32], F32)
    m_b = nc.alloc_sbuf_tensor("m_b", [BL, 32], F32)
    a1 = nc.alloc_sbuf_tensor("a1", [BL, 16], F32)
    a2 = nc.alloc_sbuf_tensor("a2", [BL, 64], F32)
    wa = nc.alloc_sbuf_tensor("wa", [BL, P], F32)
    b1 = nc.alloc_sbuf_tensor("b1", [BL, 16], F32)
    b2 = nc.alloc_sbuf_tensor("b2", [BL, 64], F32)
    wb = nc.alloc_sbuf_tensor("wb", [BL, P], F32)
    waT0 = nc.alloc_sbuf_tensor("waT0", [128, BL], BF16)
    waT1 = nc.alloc_sbuf_tensor("waT1", [128, BL], BF16)
    w_sb0 = nc.alloc_sbuf_tensor("w_sb0", [128, NW], BF16)
    w_sb1 = nc.alloc_sbuf_tensor("w_sb1", [128, NW], BF16)
    tA = nc.alloc_sbuf_tensor("tA", [BL, P], F32)
    tA2 = nc.alloc_sbuf_tensor("tA2", [BL, P], F32)
    tB = nc.alloc_sbuf_tensor("tB", [BL, P], F32)
    tB2 = nc.alloc_sbuf_tensor("tB2", [BL, P], F32)
    rF = nc.alloc_sbuf_tensor("rF", [BL, P], F32)
    rF2 = nc.alloc_sbuf_tensor("rF2", [BL, P], F32)
    p_scr = nc.alloc_sbuf_tensor("p_scr", [BL, P], F32)
    swa = nc.alloc_sbuf_tensor("swa", [BL, 1], F32)
    swb = nc.alloc_sbuf_tensor("swb", [BL, 1], F32)
    den = nc.alloc_sbuf_tensor("den", [BL, 1], F32)
    den2 = nc.alloc_sbuf_tensor("den2", [BL, 1], F32)
    rec = nc.alloc_sbuf_tensor("rec", [BL, 1], F32)
    num = nc.alloc_sbuf_tensor("num", [BL, 1], F32)
    outv = nc.alloc_sbuf_tensor("outv", [BL, 1], F32)

    # PSUM: 4x (128,512) + 1x (128,256) Z chunks + 2 transpose outputs
    n_slices = [(0, 512), (512, 1024), (1024, 1536), (1536, 2048), (2048, NW)]
    z_ps = [
        nc.alloc_psum_tensor(f"z{c}", [BL, n1 - n0], F32)
        for c, (n0, n1) in enumerate(n_slices)
    ]
    t0_ps = nc.alloc_psum_tensor("t0_ps", [128, BL], F32)
    t1_ps = nc.alloc_psum_tensor("t1_ps", [128, BL], F32)

    mult = mybir.AluOpType.mult
    add = mybir.AluOpType.add
    sub = mybir.AluOpType.subtract
    AF = mybir.ActivationFunctionType

    with (
        nc.Block() as block,
        nc.semaphore("dsem") as dsem,    # small DMAs (x, mu, sig, ident)
        nc.semaphore("wsem0") as wsem0,  # weight-chunk DMAs (one sem/chunk)
        nc.semaphore("wsem1") as wsem1,
        nc.semaphore("wsem2") as wsem2,
        nc.semaphore("wsem3") as wsem3,
        nc.semaphore("wsem4") as wsem4,
        nc.semaphore("vsem") as vsem,    # DVE -> other engines
        nc.semaphore("ssem") as ssem,    # ACT -> other engines
        nc.semaphore("tsem") as tsem,    # PE  -> other engines
        nc.semaphore("vv") as vv,        # DVE same-engine chain
        nc.semaphore("sv") as sv,        # ACT same-engine chain
        nc.semaphore("osem") as osem,    # output DMA
    ):
        wsems = [wsem0, wsem1, wsem2, wsem3, wsem4]

        # ---------------- SP: big weight DMAs (own HWDGE ring) ----------
        @block.sync
        def _(sync):
            for c, (n0, n1) in enumerate(n_slices):
                sync.dma_start(
                    out=w_sb0[:, n0:n1], in_=w_e[0, :, n0:n1]
                ).then_inc(wsems[c], 16)
                sync.dma_start(
                    out=w_sb1[:, n0:n1], in_=w_e[1, :, n0:n1]
                ).then_inc(wsems[c], 16)

        # ---------------- ScalarE: small DMAs + activations + copies ----
        @block.scalar
        def _(scalar):
            scalar.dma_start(out=x_sb[:], in_=x_e[:]).then_inc(dsem, 16)
            mu_src = musig_e[0:1, :].partition_broadcast(BL).squeeze(1)
            scalar.dma_start(out=mu_b[:], in_=mu_src).then_inc(dsem, 16)
            sig_src = musig_e[1:2, :].partition_broadcast(BL).squeeze(1)
            scalar.dma_start(out=sig_b[:], in_=sig_src).then_inc(dsem, 16)
            scalar.dma_start(out=id_sb[:], in_=id_e[:]).then_inc(dsem, 16)

            scalar.wait_ge(dsem, 64)  # x, mu, sig, ident landed
            scalar.activation(sabs[:], sig_b[:], AF.Abs).then_inc(sv, 1)
            scalar.wait_ge(sv, 1)
            # v2 = (|sigma| + 1e-5)^2
            scalar.activation(v2_b[:], sabs[:], AF.Square, bias=1e-5).then_inc(
                ssem, 1
            )
            scalar.wait_ge(vsem, 1)   # e2 = dsq * rc ready
            # m = exp(-0.5 * e2)
            scalar.activation(m_b[:], e_b[:], AF.Exp, scale=-0.5).then_inc(ssem, 1)
            scalar.wait_ge(tsem, 1)
            scalar.copy(waT0[:], t0_ps[:]).then_inc(ssem, 1)
            scalar.wait_ge(tsem, 2)
            scalar.copy(waT1[:], t1_ps[:]).then_inc(ssem, 1)

        # ---------------- TensorE: transposes + the big matmul ----------
        @block.tensor
        def _(tensor):
            tensor.wait_ge(dsem, 64)  # ident landed
            tensor.wait_ge(vsem, 2)   # wa built
            tensor.transpose(t0_ps[:], wa[:, 0:128], id_sb[:]).then_inc(tsem, 1)
            tensor.transpose(t1_ps[:], wa[:, 128:256], id_sb[:]).then_inc(tsem, 1)
            tensor.wait_ge(ssem, 4)   # waT0/waT1 copied to SBUF
            for c, (n0, n1) in enumerate(n_slices):
                tensor.wait_ge(wsems[c], 32)
                tensor.matmul(
                    z_ps[c][:], waT0[:], w_sb0[:, n0:n1], start=True, stop=False
                )
                tensor.matmul(
                    z_ps[c][:], waT1[:], w_sb1[:, n0:n1], start=False, stop=True
                ).then_inc(tsem, 1)

        # ---------------- VectorE: memberships, outers, epilogue --------
        @block.vector
        def _(vector):
            vc = [0]  # vv chain counter

            def step(ins, waits=()):
                for w in waits:
                    vector.wait_ge(vv, w)
                ins().then_inc(vv, 1)
                vc[0] += 1
                return vc[0]

            def outer(out_t, a_ap, b_ap, na, nb):
                in0 = a_ap.unsqueeze(2).to_broadcast((BL, na, nb))
                in1 = b_ap.unsqueeze(1).to_broadcast((BL, na, nb))
                o3 = out_t[:].rearrange("p (a b) -> p a b", b=nb)
                return vector.tensor_tensor(o3, in0, in1, mult)

            x_bc = x_sb[:].unsqueeze(2).to_broadcast((BL, NI, NMF))
            mu3 = mu_b[:].rearrange("p (i k) -> p i k", k=NMF)
            d3 = d_b[:].rearrange("p (i k) -> p i k", k=NMF)

            vector.wait_ge(dsem, 64)
            i_d = step(lambda: vector.tensor_tensor(d3, x_bc, mu3, sub))
            i_dsq = step(
                lambda: vector.tensor_tensor(dsq_b[:], d_b[:], d_b[:], mult),
                waits=[i_d],
            )
            vector.wait_ge(ssem, 1)  # v2 ready
            i_rc = step(lambda: vector.reciprocal(rc_b[:], v2_b[:]))
            # e2 = dsq * rc  (ScalarE applies exp(-0.5 * e2))
            vector.wait_ge(vv, i_rc)
            vector.tensor_tensor(e_b[:], dsq_b[:], rc_b[:], mult).then_inc(vsem, 1)

            vector.wait_ge(ssem, 2)  # m_b ready
            i_a1 = step(lambda: outer(a1, m_b[:, 0:4], m_b[:, 4:8], 4, 4))
            i_b1 = step(lambda: outer(b1, m_b[:, 16:20], m_b[:, 20:24], 4, 4))
            i_a2 = step(lambda: outer(a2, a1[:], m_b[:, 8:12], 16, 4), waits=[i_a1])
            i_b2 = step(lambda: outer(b2, b1[:], m_b[:, 24:28], 16, 4), waits=[i_b1])
            # wa signals cross-engine (PE transposes) via vsem=2
            vector.wait_ge(vv, i_a2)
            vector.tensor_tensor(
                wa[:].rearrange("p (a b) -> p a b", b=4),
                a2[:].unsqueeze(2).to_broadcast((BL, 64, 4)),
                m_b[:, 12:16].unsqueeze(1).to_broadcast((BL, 64, 4)),
                mult,
            ).then_inc(vsem, 1)
            i_wb = step(
                lambda: outer(wb, b2[:], m_b[:, 28:32], 64, 4), waits=[i_b2]
            )
            vector.wait_ge(vsem, 2)
            i_swa = step(
                lambda: vector.reduce_sum(swa[:], wa[:], axis=mybir.AxisListType.X)
            )
            i_swb = step(
                lambda: vector.reduce_sum(swb[:], wb[:], axis=mybir.AxisListType.X),
                waits=[i_wb],
            )
            i_den = step(
                lambda: vector.tensor_tensor(den[:], swa[:], swb[:], mult),
                waits=[i_swa, i_swb],
            )
            i_den2 = step(
                lambda: vector.tensor_scalar_add(den2[:], den[:], 1e-6),
                waits=[i_den],
            )
            i_rec = step(lambda: vector.reciprocal(rec[:], den2[:]), waits=[i_den2])

            # epilogue: R = sum_j x9_j * Z_j over two interleaved chains
            xc = [x_sb[:, j : j + 1] for j in range(NI)]
            zblk = []
            for c in range(4):
                zblk.append(z_ps[c][:, 0:P])
                zblk.append(z_ps[c][:, P : 2 * P])
            zblk.append(z_ps[4][:, 0:P])

            vector.wait_ge(tsem, 3)  # chunk 0 (j=0,1)
            i_tA = step(lambda: vector.tensor_scalar_mul(tA[:], zblk[0], xc[0]))
            i_tB = step(lambda: vector.tensor_scalar_mul(tB[:], zblk[1], xc[1]))
            vector.wait_ge(tsem, 4)  # chunk 1 (j=2,3)
            i_tA2 = step(
                lambda: vector.scalar_tensor_tensor(
                    tA2[:], zblk[2], xc[2], tA[:], mult, add
                ),
                waits=[i_tA],
            )
            i_tB2 = step(
                lambda: vector.scalar_tensor_tensor(
                    tB2[:], zblk[3], xc[3], tB[:], mult, add
                ),
                waits=[i_tB],
            )
            vector.wait_ge(tsem, 5)  # chunk 2 (j=4,5)
            i_tA = step(
                lambda: vector.scalar_tensor_tensor(
                    tA[:], zblk[4], xc[4], tA2[:], mult, add
                ),
                waits=[i_tA2],
            )
            i_tB = step(
                lambda: vector.scalar_tensor_tensor(
                    tB[:], zblk[5], xc[5], tB2[:], mult, add
                ),
                waits=[i_tB2],
            )
            vector.wait_ge(tsem, 6)  # chunk 3 (j=6,7)
            i_tA2 = step(
                lambda: vector.scalar_tensor_tensor(
                    tA2[:], zblk[6], xc[6], tA[:], mult, add
                ),
                waits=[i_tA],
            )
            i_tB2 = step(
                lambda: vector.scalar_tensor_tensor(
                    tB2[:], zblk[7], xc[7], tB[:], mult, add
                ),
                waits=[i_tB],
            )
            i_rF = step(
                lambda: vector.tensor_tensor(rF[:], tA2[:], tB2[:], add),
                waits=[i_tA2, i_tB2],
            )
            vector.wait_ge(tsem, 7)  # chunk 4 (j=8, bias)
            i_rF2 = step(
                lambda: vector.tensor_tensor(rF2[:], zblk[8], rF[:], add),
                waits=[i_rF],
            )
            i_p = step(
                lambda: vector.tensor_tensor(p_scr[:], rF2[:], wb[:], mult),
                waits=[i_rF2],
            )
            i_num = step(
                lambda: vector.reduce_sum(
                    num[:], p_scr[:], axis=mybir.AxisListType.X
                ),
                waits=[i_p],
            )
            vector.wait_ge(vv, i_num)
            vector.wait_ge(vv, i_rec)
            vector.tensor_tensor(outv[:], num[:], rec[:], mult).then_inc(vsem, 1)

        # ---------------- GpSimd: output DMA -----------------------------
        dbg_specs = []
        if debug:
            for t, shape in [
                (m_b, [BL, 32]),
                (wa, [BL, P]),
                (wb, [BL, P]),
                (rF2, [BL, P]),
                (swa, [BL, 1]),
                (den2, [BL, 1]),
                (num, [BL, 1]),
            ]:
                d_e = nc.declare_dram_parameter(
                    "dbg_" + t.name, shape, F32, isOutput=True
                )
                dbg_specs.append((d_e, t))

        @block.gpsimd
        def _(gpsimd):
            gpsimd.wait_ge(vsem, 3)
            gpsimd.dma_start(out=out_e[:], in_=outv[:]).then_inc(osem, 16)
            n_dbg = len(dbg_specs)
            for d_e, t in dbg_specs:
                gpsimd.dma_start(out=d_e[:], in_=t[:]).then_inc(osem, 16)
            gpsimd.wait_ge(osem, 16 * (1 + n_dbg))

    return nc


_CACHE = {}


def _get_nc():
    if "nc" not in _CACHE:
        _CACHE["nc"] = build_nc()
    return _CACHE["nc"]


def _prep_in_maps(x, mu, sigma, consequent_weights, consequent_bias):
    import ml_dtypes

    x = np.ascontiguousarray(np.asarray(x, dtype=np.float32))
    mu = np.asarray(mu, dtype=np.float32)
    sigma = np.asarray(sigma, dtype=np.float32)
    cw = np.asarray(consequent_weights, dtype=np.float32)
    cb = np.asarray(consequent_bias, dtype=np.float32)

    musig = np.stack([mu.reshape(32), sigma.reshape(32)]).astype(np.float32)
    # W[p, j*256+q] = V9[p, q, j]
    v9 = np.concatenate(
        [cw.reshape(P, P, NI), cb.reshape(P, P, 1)], axis=2
    )  # (p, q, j)
    wmat = np.ascontiguousarray(v9.transpose(0, 2, 1)).reshape(P, NW)
    wmat = np.ascontiguousarray(wmat.reshape(2, 128, NW)).astype(ml_dtypes.bfloat16)
    ident = np.eye(128, dtype=np.float32)

    in_maps = []
    for c in range(N_CORES):
        in_maps.append(
            {
                "x": np.ascontiguousarray(x[c * BL : (c + 1) * BL]),
                "musig": musig,
                "wmat": wmat,
                "ident": ident,
            }
        )
    return in_maps


def run(inputs: dict, trace: bool = False):
    nc = _get_nc()
    in_maps = _prep_in_maps(**inputs)
    res = run_bass_kernel_spmd(
        nc,
        in_maps,
        core_ids=list(range(N_CORES)),
        trace=trace,
        trace_cores=list(range(N_CORES)) if trace else None,
    )
    out = np.concatenate([res.results[c]["out"] for c in range(N_CORES)], axis=0)
    return out.astype(np.float32), res


def kernel(**inputs) -> np.ndarray:
    out, _ = run(inputs, trace=False)
    return out


# revision 19
# speedup vs baseline: 1.0079x; 1.0079x over previous
"""ANFIS layer kernel for 8 TRN2 NeuronCores.

Math: the rule-strength tensor w[b, r] (B x 65536) is a Kronecker product of
8 per-input membership vectors (4 each).  Split rules r = p*256 + q with
p = digits of inputs 0-3, q = digits of inputs 4-7:
    w[b, p*256+q] = wa[b,p] * wb[b,q]
so with V9[p, q, j] = [consequent_weights | consequent_bias][p*256+q, j]:
    num[b]  = sum_j x9[b,j] * (wa[b]^T V9[:,:,j] wb[b])      (x9 = [x, 1])
    wsum[b] = sum(wa[b]) * sum(wb[b])
    out[b]  = num[b] / (wsum[b] + 1e-6)
The 65536-wide tensor is never materialized; per core the heavy op is a
(128 x 256) @ (256 x 2304) bf16 matmul.  Batch is sharded across the 8 cores.

Sync discipline: EVERY read-after-write needs a semaphore edge, including
same-engine consecutive ops (verified on HW: unsynced same-engine RAW reads
stale data), and outputs must not alias inputs.
"""

import numpy as np

import concourse.bass as bass
import concourse.mybir as mybir
from concourse.bass_utils import run_bass_kernel_spmd

N_CORES = 8
B = 1024
BL = B // N_CORES  # 128 batch rows per core
NI = 8             # inputs
NMF = 4            # membership functions per input
P = NMF ** 4       # 256
NJ = NI + 1        # 8 weight columns + bias
NW = NJ * P        # 2304 output columns of the matmul

F32 = mybir.dt.float32
BF16 = mybir.dt.bfloat16


def build_nc(debug: bool = False):
    nc = bass.Bass(trn_type="TRN2")

    x_e = nc.declare_dram_parameter("x", [BL, NI], F32, isOutput=False)
    musig_e = nc.declare_dram_parameter("musig", [2, 32], F32, isOutput=False)
    w_e = nc.declare_dram_parameter("wmat", [2, 128, NW], BF16, isOutput=False)
    id_e = nc.declare_dram_parameter("ident", [128, 128], BF16, isOutput=False)
    out_e = nc.declare_dram_parameter("out", [BL, 1], F32, isOutput=True)

    # SBUF
    x_sb = nc.alloc_sbuf_tensor("x_sb", [BL, NI], F32)
    id_sb = nc.alloc_sbuf_tensor("id_sb", [128, 128], BF16)
    ms_b = nc.alloc_sbuf_tensor("ms_b", [BL, 64], F32)   # [mu | sigma] bcast
    sabs = nc.alloc_sbuf_tensor("sabs", [BL, 32], F32)
    u_b = nc.alloc_sbuf_tensor("u_b", [BL, 32], F32)
    v_b = nc.alloc_sbuf_tensor("v_b", [BL, 32], F32)
    rc_b = nc.alloc_sbuf_tensor("rc_b", [BL, 32], F32)
    d_b = nc.alloc_sbuf_tensor("d_b", [BL, 32], F32)
    dsq_b = nc.alloc_sbuf_tensor("dsq_b", [BL, 32], F32)
    e_b = nc.alloc_sbuf_tensor("e_b", [BL, 32], F32)
    m_b = nc.alloc_sbuf_tensor("m_b", [BL, 32], BF16)
    a1 = nc.alloc_sbuf_tensor("a1", [BL, 16], BF16)
    a2 = nc.alloc_sbuf_tensor("a2", [BL, 64], BF16)
    wa = nc.alloc_sbuf_tensor("wa", [BL, P], BF16)
    b1 = nc.alloc_sbuf_tensor("b1", [BL, 16], BF16)
    b2 = nc.alloc_sbuf_tensor("b2", [BL, 64], BF16)
    wb = nc.alloc_sbuf_tensor("wb", [BL, P], BF16)
    waT0 = nc.alloc_sbuf_tensor("waT0", [128, BL], BF16)
    waT1 = nc.alloc_sbuf_tensor("waT1", [128, BL], BF16)
    w_sb0 = nc.alloc_sbuf_tensor("w_sb0", [128, NW], BF16)
    w_sb1 = nc.alloc_sbuf_tensor("w_sb1", [128, NW], BF16)
    tA = nc.alloc_sbuf_tensor("tA", [BL, P], F32)
    tA2 = nc.alloc_sbuf_tensor("tA2", [BL, P], F32)
    tB = nc.alloc_sbuf_tensor("tB", [BL, P], F32)
    tB2 = nc.alloc_sbuf_tensor("tB2", [BL, P], F32)
    rF = nc.alloc_sbuf_tensor("rF", [BL, P], F32)
    rF2 = nc.alloc_sbuf_tensor("rF2", [BL, P], F32)
    p_scr = nc.alloc_sbuf_tensor("p_scr", [BL, P], F32)
    swa = nc.alloc_sbuf_tensor("swa", [BL, 1], F32)
    swb = nc.alloc_sbuf_tensor("swb", [BL, 1], F32)
    den = nc.alloc_sbuf_tensor("den", [BL, 1], F32)
    den2 = nc.alloc_sbuf_tensor("den2", [BL, 1], F32)
    rec = nc.alloc_sbuf_tensor("rec", [BL, 1], F32)
    num = nc.alloc_sbuf_tensor("num", [BL, 1], F32)
    outv = nc.alloc_sbuf_tensor("outv", [BL, 1], F32)

    # PSUM: 4x (128,512) + 1x (128,256) Z chunks + 2 transpose outputs
    n_slices = [(0, 512), (512, 1024), (1024, 1536), (1536, 2048), (2048, NW)]
    z_ps = [
        nc.alloc_psum_tensor(f"z{c}", [BL, n1 - n0], F32)
        for c, (n0, n1) in enumerate(n_slices)
    ]
    t0_ps = nc.alloc_psum_tensor("t0_ps", [128, BL], BF16)
    t1_ps = nc.alloc_psum_tensor("t1_ps", [128, BL], BF16)

    mult = mybir.AluOpType.mult
    add = mybir.AluOpType.add
    sub = mybir.AluOpType.subtract
    AF = mybir.ActivationFunctionType

    with (
        nc.Block() as block,
        nc.semaphore("dsx") as dsx,      # x DMA
        nc.semaphore("dsm") as dsm,      # musig bcast DMA
        nc.semaphore("dsi") as dsi,      # ident DMA
        nc.semaphore("wsA") as wsA,      # W half 0 DMA
        nc.semaphore("wsB") as wsB,      # W half 1 DMA
        nc.semaphore("vsem") as vsem,    # DVE -> other engines
        nc.semaphore("ssem") as ssem,    # ACT -> other engines
        nc.semaphore("tsem") as tsem,    # PE  -> other engines
        nc.semaphore("vv") as vv,        # DVE same-engine chain
        nc.semaphore("sv") as sv,        # ACT same-engine chain
        nc.semaphore("osem") as osem,    # output DMA
    ):
        # ---------------- SP: the two big weight DMAs -------------------
        @block.sync
        def _(sync):
            sync.dma_start(out=w_sb0[:], in_=w_e[0]).then_inc(wsA, 16)
            sync.dma_start(out=w_sb1[:], in_=w_e[1]).then_inc(wsB, 16)

        # ---------------- ScalarE: small DMAs + activations + copies ----
        @block.scalar
        def _(scalar):
            scalar.dma_start(out=x_sb[:], in_=x_e[:]).then_inc(dsx, 16)
            ms_src = (
                musig_e[:]
                .rearrange("r c -> (r c)")
                .unsqueeze(0)
                .partition_broadcast(BL)
                .squeeze(1)
            )
            scalar.dma_start(out=ms_b[:], in_=ms_src).then_inc(dsm, 16)
            scalar.dma_start(out=id_sb[:], in_=id_e[:]).then_inc(dsi, 16)

            scalar.wait_ge(dsm, 16)
            scalar.activation(sabs[:], ms_b[:, 32:64], AF.Abs).then_inc(ssem, 1)
            scalar.wait_ge(vsem, 1)   # e2 = dsq * rc ready
            # m = exp(-0.5 * e2), rounded to bf16
            scalar.activation(m_b[:], e_b[:], AF.Exp, scale=-0.5).then_inc(ssem, 1)
            scalar.wait_ge(tsem, 1)
            scalar.copy(waT0[:], t0_ps[:]).then_inc(ssem, 1)
            scalar.wait_ge(tsem, 2)
            scalar.copy(waT1[:], t1_ps[:]).then_inc(ssem, 1)
            # output DMA once DVE finishes
            scalar.wait_ge(vsem, 3)
            scalar.dma_start(out=out_e[:], in_=outv[:]).then_inc(osem, 16)
            scalar.wait_ge(osem, 16)

        # ---------------- TensorE: transposes + the big matmul ----------
        @block.tensor
        def _(tensor):
            tensor.wait_ge(dsi, 16)   # ident landed
            tensor.wait_ge(vsem, 2)   # wa built
            tensor.transpose(t0_ps[:], wa[:, 0:128], id_sb[:]).then_inc(tsem, 1)
            tensor.transpose(t1_ps[:], wa[:, 128:256], id_sb[:]).then_inc(tsem, 1)
            tensor.wait_ge(ssem, 3)   # waT0 in SBUF
            tensor.wait_ge(wsA, 16)   # W half 0 in SBUF
            for c, (n0, n1) in enumerate(n_slices):
                tensor.matmul(
                    z_ps[c][:], waT0[:], w_sb0[:, n0:n1], start=True, stop=False
                )
            tensor.wait_ge(ssem, 4)   # waT1 in SBUF
            tensor.wait_ge(wsB, 16)   # W half 1 in SBUF
            for c, (n0, n1) in enumerate(n_slices):
                tensor.matmul(
                    z_ps[c][:], waT1[:], w_sb1[:, n0:n1], start=False, stop=True
                ).then_inc(tsem, 1)

        # ---------------- VectorE: memberships, outers, epilogue --------
        @block.vector
        def _(vector):
            vc = [0]  # vv chain counter

            def step(ins, waits=()):
                for w in waits:
                    vector.wait_ge(vv, w)
                ins().then_inc(vv, 1)
                vc[0] += 1
                return vc[0]

            def outer(out_t, a_ap, b_ap, na, nb):
                in0 = a_ap.unsqueeze(2).to_broadcast((BL, na, nb))
                in1 = b_ap.unsqueeze(1).to_broadcast((BL, na, nb))
                o3 = out_t[:].rearrange("p (a b) -> p a b", b=nb)
                return vector.tensor_tensor(o3, in0, in1, mult)

            x_bc = x_sb[:].unsqueeze(2).to_broadcast((BL, NI, NMF))
            mu3 = ms_b[:, 0:32].rearrange("p (i k) -> p i k", k=NMF)
            d3 = d_b[:].rearrange("p (i k) -> p i k", k=NMF)

            vector.wait_ge(dsx, 16)
            vector.wait_ge(dsm, 16)
            i_d = step(lambda: vector.tensor_tensor(d3, x_bc, mu3, sub))
            i_dsq = step(
                lambda: vector.tensor_tensor(dsq_b[:], d_b[:], d_b[:], mult),
                waits=[i_d],
            )
            vector.wait_ge(ssem, 1)  # sabs ready
            i_u = step(lambda: vector.tensor_scalar_add(u_b[:], sabs[:], 1e-5))
            i_v = step(
                lambda: vector.tensor_tensor(v_b[:], u_b[:], u_b[:], mult),
                waits=[i_u],
            )
            i_rc = step(lambda: vector.reciprocal(rc_b[:], v_b[:]), waits=[i_v])
            # e2 = dsq * rc  (ScalarE applies exp(-0.5 * e2))
            vector.wait_ge(vv, i_rc)
            vector.tensor_tensor(e_b[:], dsq_b[:], rc_b[:], mult).then_inc(vsem, 1)

            vector.wait_ge(ssem, 2)  # m_b ready
            i_a1 = step(lambda: outer(a1, m_b[:, 0:4], m_b[:, 4:8], 4, 4))
            i_b1 = step(lambda: outer(b1, m_b[:, 16:20], m_b[:, 20:24], 4, 4))
            i_a2 = step(lambda: outer(a2, a1[:], m_b[:, 8:12], 16, 4), waits=[i_a1])
            i_b2 = step(lambda: outer(b2, b1[:], m_b[:, 24:28], 16, 4), waits=[i_b1])
            # wa signals cross-engine (PE transposes) via vsem=2
            vector.wait_ge(vv, i_a2)
            vector.tensor_tensor(
                wa[:].rearrange("p (a b) -> p a b", b=4),
                a2[:].unsqueeze(2).to_broadcast((BL, 64, 4)),
                m_b[:, 12:16].unsqueeze(1).to_broadcast((BL, 64, 4)),
                mult,
            ).then_inc(vsem, 1)
            i_wb = step(
                lambda: outer(wb, b2[:], m_b[:, 28:32], 64, 4), waits=[i_b2]
            )
            vector.wait_ge(vsem, 2)
            i_swa = step(
                lambda: vector.reduce_sum(swa[:], wa[:], axis=mybir.AxisListType.X)
            )
            i_swb = step(
                lambda: vector.reduce_sum(swb[:], wb[:], axis=mybir.AxisListType.X),
                waits=[i_wb],
            )
            i_den = step(
                lambda: vector.tensor_tensor(den[:], swa[:], swb[:], mult),
                waits=[i_swa, i_swb],
            )
            i_den2 = step(
                lambda: vector.tensor_scalar_add(den2[:], den[:], 1e-6),
                waits=[i_den],
            )
            i_rec = step(lambda: vector.reciprocal(rec[:], den2[:]), waits=[i_den2])

            # epilogue: R = sum_j x9_j * Z_j over two interleaved chains
            xc = [x_sb[:, j : j + 1] for j in range(NI)]
            zblk = []
            for c in range(4):
                zblk.append(z_ps[c][:, 0:P])
                zblk.append(z_ps[c][:, P : 2 * P])
            zblk.append(z_ps[4][:, 0:P])

            vector.wait_ge(tsem, 3)  # chunk 0 (j=0,1)
            i_tA = step(lambda: vector.tensor_scalar_mul(tA[:], zblk[0], xc[0]))
            i_tB = step(lambda: vector.tensor_scalar_mul(tB[:], zblk[1], xc[1]))
            vector.wait_ge(tsem, 4)  # chunk 1 (j=2,3)
            i_tA2 = step(
                lambda: vector.scalar_tensor_tensor(
                    tA2[:], zblk[2], xc[2], tA[:], mult, add
                ),
                waits=[i_tA],
            )
            i_tB2 = step(
                lambda: vector.scalar_tensor_tensor(
                    tB2[:], zblk[3], xc[3], tB[:], mult, add
                ),
                waits=[i_tB],
            )
            vector.wait_ge(tsem, 5)  # chunk 2 (j=4,5)
            i_tA = step(
                lambda: vector.scalar_tensor_tensor(
                    tA[:], zblk[4], xc[4], tA2[:], mult, add
                ),
                waits=[i_tA2],
            )
            i_tB = step(
                lambda: vector.scalar_tensor_tensor(
                    tB[:], zblk[5], xc[5], tB2[:], mult, add
                ),
                waits=[i_tB2],
            )
            vector.wait_ge(tsem, 6)  # chunk 3 (j=6,7)
            i_tA2 = step(
                lambda: vector.scalar_tensor_tensor(
                    tA2[:], zblk[6], xc[6], tA[:], mult, add
                ),
                waits=[i_tA],
            )
            i_tB2 = step(
                lambda: vector.scalar_tensor_tensor(
                    tB2[:], zblk[7], xc[7], tB[:], mult, add
                ),
                waits=[i_tB],
            )
            i_rF = step(
                lambda: vector.tensor_tensor(rF[:], tA2[:], tB2[:], add),
                waits=[i_tA2, i_tB2],
            )
            vector.wait_ge(tsem, 7)  # chunk 4 (j=8, bias)
            i_rF2 = step(
                lambda: vector.tensor_tensor(rF2[:], zblk[8], rF[:], add),
                waits=[i_rF],
            )
            i_p = step(
                lambda: vector.tensor_tensor(p_scr[:], rF2[:], wb[:], mult),
                waits=[i_rF2],
            )
            i_num = step(
                lambda: vector.reduce_sum(
                    num[:], p_scr[:], axis=mybir.AxisListType.X
                ),
                waits=[i_p],
            )
            vector.wait_ge(vv, i_num)
            vector.wait_ge(vv, i_rec)
            vector.tensor_tensor(outv[:], num[:], rec[:], mult).then_inc(vsem, 1)

        # debug dumps ride on the DVE's vsem=3 signal, DMA'd from gpsimd
        dbg_specs = []
        if debug:
            for t, shape, dt in [
                (m_b, [BL, 32], BF16),
                (wa, [BL, P], BF16),
                (wb, [BL, P], BF16),
                (rF2, [BL, P], F32),
                (swa, [BL, 1], F32),
                (den2, [BL, 1], F32),
                (num, [BL, 1], F32),
            ]:
                d_e = nc.declare_dram_parameter(
                    "dbg_" + t.name, shape, dt, isOutput=True
                )
                dbg_specs.append((d_e, t))

            @block.gpsimd
            def _(gpsimd):
                gpsimd.wait_ge(vsem, 3)
                for i, (d_e, t) in enumerate(dbg_specs):
                    gpsimd.dma_start(out=d_e[:], in_=t[:]).then_inc(osem, 16)
                gpsimd.wait_ge(osem, 16 * (1 + len(dbg_specs)))

    return nc


_CACHE = {}


def _get_nc():
    if "nc" not in _CACHE:
        _CACHE["nc"] = build_nc()
    return _CACHE["nc"]


def _prep_in_maps(x, mu, sigma, consequent_weights, consequent_bias):
    import ml_dtypes

    x = np.ascontiguousarray(np.asarray(x, dtype=np.float32))
    mu = np.asarray(mu, dtype=np.float32)
    sigma = np.asarray(sigma, dtype=np.float32)
    cw = np.asarray(consequent_weights, dtype=np.float32)
    cb = np.asarray(consequent_bias, dtype=np.float32)

    musig = np.stack([mu.reshape(32), sigma.reshape(32)]).astype(np.float32)
    # W[p, j*256+q] = V9[p, q, j]
    v9 = np.concatenate(
        [cw.reshape(P, P, NI), cb.reshape(P, P, 1)], axis=2
    )  # (p, q, j)
    wmat = np.ascontiguousarray(v9.transpose(0, 2, 1)).reshape(P, NW)
    wmat = np.ascontiguousarray(wmat.reshape(2, 128, NW)).astype(ml_dtypes.bfloat16)
    ident = np.eye(128, dtype=np.float32).astype(ml_dtypes.bfloat16)

    in_maps = []
    for c in range(N_CORES):
        in_maps.append(
            {
                "x": np.ascontiguousarray(x[c * BL : (c + 1) * BL]),
                "musig": musig,
                "wmat": wmat,
                "ident": ident,
            }
        )
    return in_maps


def run(inputs: dict, trace: bool = False):
    nc = _get_nc()
    in_maps = _prep_in_maps(**inputs)
    res = run_bass_kernel_spmd(
        nc,
        in_maps,
        core_ids=list(range(N_CORES)),
        trace=trace,
        trace_cores=list(range(N_CORES)) if trace else None,
    )
    out = np.concatenate([res.results[c]["out"] for c in range(N_CORES)], axis=0)
    return out.astype(np.float32), res


def kernel(**inputs) -> np.ndarray:
    out, _ = run(inputs, trace=False)
    return out


# revision 21
# speedup vs baseline: 1.0581x; 1.0498x over previous
"""ANFIS layer kernel for 8 TRN2 NeuronCores.

Math: the rule-strength tensor w[b, r] (B x 65536) is a Kronecker product of
8 per-input membership vectors (4 each).  Split rules r = p*256 + q with
p = digits of inputs 0-3, q = digits of inputs 4-7:
    w[b, p*256+q] = wa[b,p] * wb[b,q]
so with V9[p, q, j] = [consequent_weights | consequent_bias][p*256+q, j]:
    num[b]  = sum_j x9[b,j] * (wa[b]^T V9[:,:,j] wb[b])      (x9 = [x, 1])
    wsum[b] = sum(wa[b]) * sum(wb[b])
    out[b]  = num[b] / (wsum[b] + 1e-6)
The 65536-wide tensor is never materialized; per core the heavy op is a
(128 x 256) @ (256 x 2304) bf16 matmul.  Batch is sharded across the 8 cores.

Sync discipline: EVERY read-after-write needs a semaphore edge, including
same-engine consecutive ops (verified on HW: unsynced same-engine RAW reads
stale data), and outputs must not alias inputs.
"""

import numpy as np

import concourse.bass as bass
import concourse.mybir as mybir
from concourse.bass_utils import run_bass_kernel_spmd

N_CORES = 8
B = 1024
BL = B // N_CORES  # 128 batch rows per core
NI = 8             # inputs
NMF = 4            # membership functions per input
P = NMF ** 4       # 256
NJ = NI + 1        # 8 weight columns + bias
NW = NJ * P        # 2304 output columns of the matmul

F32 = mybir.dt.float32
BF16 = mybir.dt.bfloat16

N_WARM_MM = 16     # dummy matmuls to lift the PE clock gate before real work


def build_nc(debug: bool = False):
    nc = bass.Bass(trn_type="TRN2")

    # const AP for the sigma epsilon (activation bias)
    eps_t = nc.alloc_sbuf_tensor("const-eps5", [128, 1], F32)
    nc.gpsimd.memset(eps_t.ap(), 1e-5)
    nc.const_aps.aps[(F32, 1e-5)] = eps_t.ap()
    nc.all_engine_barrier()

    xms_e = nc.declare_dram_parameter("xms", [BL, 72], F32, isOutput=False)
    w_e = nc.declare_dram_parameter("wmat", [2, 128, NW], BF16, isOutput=False)
    id_e = nc.declare_dram_parameter("ident", [128, 128], BF16, isOutput=False)
    out_e = nc.declare_dram_parameter("out", [BL, 1], F32, isOutput=True)

    # SBUF
    xms = nc.alloc_sbuf_tensor("xms_sb", [BL, 72], F32)  # [x | mu | sigma]
    id_sb = nc.alloc_sbuf_tensor("id_sb", [128, 128], BF16)
    v2_b = nc.alloc_sbuf_tensor("v2_b", [BL, 32], F32)
    rc_b = nc.alloc_sbuf_tensor("rc_b", [BL, 32], F32)
    d_b = nc.alloc_sbuf_tensor("d_b", [BL, 32], F32)
    dsq_b = nc.alloc_sbuf_tensor("dsq_b", [BL, 32], F32)
    e_b = nc.alloc_sbuf_tensor("e_b", [BL, 32], F32)
    m_b = nc.alloc_sbuf_tensor("m_b", [BL, 32], BF16)
    a1 = nc.alloc_sbuf_tensor("a1", [BL, 16], BF16)
    a2 = nc.alloc_sbuf_tensor("a2", [BL, 64], BF16)
    wa = nc.alloc_sbuf_tensor("wa", [BL, P], BF16)
    b1 = nc.alloc_sbuf_tensor("b1", [BL, 16], BF16)
    b2 = nc.alloc_sbuf_tensor("b2", [BL, 64], BF16)
    wb = nc.alloc_sbuf_tensor("wb", [BL, P], BF16)
    waT0 = nc.alloc_sbuf_tensor("waT0", [128, BL], BF16)
    waT1 = nc.alloc_sbuf_tensor("waT1", [128, BL], BF16)
    w_sb0 = nc.alloc_sbuf_tensor("w_sb0", [128, NW], BF16)
    w_sb1 = nc.alloc_sbuf_tensor("w_sb1", [128, NW], BF16)
    tA = nc.alloc_sbuf_tensor("tA", [BL, P], F32)
    tA2 = nc.alloc_sbuf_tensor("tA2", [BL, P], F32)
    tB = nc.alloc_sbuf_tensor("tB", [BL, P], F32)
    tB2 = nc.alloc_sbuf_tensor("tB2", [BL, P], F32)
    rF = nc.alloc_sbuf_tensor("rF", [BL, P], F32)
    p_scr = nc.alloc_sbuf_tensor("p_scr", [BL, P], F32)
    swa = nc.alloc_sbuf_tensor("swa", [BL, 1], F32)
    swb = nc.alloc_sbuf_tensor("swb", [BL, 1], F32)
    den = nc.alloc_sbuf_tensor("den", [BL, 1], F32)
    den2 = nc.alloc_sbuf_tensor("den2", [BL, 1], F32)
    rec = nc.alloc_sbuf_tensor("rec", [BL, 1], F32)
    num = nc.alloc_sbuf_tensor("num", [BL, 1], F32)
    outv = nc.alloc_sbuf_tensor("outv", [BL, 1], F32)
    scr1 = nc.alloc_sbuf_tensor("scr1", [1, 4], F32)  # dummy act target

    # PSUM: 5 Z chunks + 2 transpose outputs + 1 warm-up scratch = 8 banks
    n_slices = [(0, 512), (512, 1024), (1024, 1536), (1536, 2048), (2048, NW)]
    z_ps = [
        nc.alloc_psum_tensor(f"z{c}", [BL, n1 - n0], F32)
        for c, (n0, n1) in enumerate(n_slices)
    ]
    t0_ps = nc.alloc_psum_tensor("t0_ps", [128, BL], BF16)
    t1_ps = nc.alloc_psum_tensor("t1_ps", [128, BL], BF16)
    warm_ps = nc.alloc_psum_tensor("warm_ps", [128, 128], F32)

    mult = mybir.AluOpType.mult
    add = mybir.AluOpType.add
    sub = mybir.AluOpType.subtract
    byp = mybir.AluOpType.bypass
    AF = mybir.ActivationFunctionType

    with (
        nc.Block() as block,
        nc.semaphore("dsm") as dsm,      # xms DMA
        nc.semaphore("dsi") as dsi,      # ident DMA
        nc.semaphore("wsA") as wsA,      # W half 0 DMA
        nc.semaphore("wsB") as wsB,      # W half 1 DMA
        nc.semaphore("vsem") as vsem,    # DVE -> other engines
        nc.semaphore("ssem") as ssem,    # ACT -> other engines
        nc.semaphore("tsem") as tsem,    # PE  -> other engines
        nc.semaphore("vv") as vv,        # DVE same-engine chain
        nc.semaphore("osem") as osem,    # output DMA
    ):
        # ---------------- SP: ident + the two big weight DMAs -----------
        @block.sync
        def _(sync):
            sync.dma_start(out=id_sb[:], in_=id_e[:]).then_inc(dsi, 16)
            sync.dma_start(out=w_sb0[:], in_=w_e[0]).then_inc(wsA, 16)
            sync.dma_start(out=w_sb1[:], in_=w_e[1]).then_inc(wsB, 16)

        # ---------------- ScalarE: xms DMA + activations + copies -------
        @block.scalar
        def _(scalar):
            scalar.dma_start(out=xms[:], in_=xms_e[:]).then_inc(dsm, 16)
            # warm the activation tables while the DMA is in flight
            scalar.activation(scr1[0:1, :], eps_t.ap()[0:1, :].to_broadcast((1, 4)), AF.Exp)
            scalar.wait_ge(dsm, 16)
            # v2 = (sigma + 1e-5)^2   (~= (|sigma|+1e-5)^2 within 2e-5 rel)
            scalar.activation(v2_b[:], xms[:, 40:72], AF.Square, bias=1e-5).then_inc(
                ssem, 1
            )
            scalar.wait_ge(vsem, 1)   # e2 = dsq * rc ready
            # m = exp(-0.5 * e2), rounded to bf16
            scalar.activation(m_b[:], e_b[:], AF.Exp, scale=-0.5).then_inc(ssem, 1)
            scalar.wait_ge(tsem, 1)
            scalar.copy(waT0[:], t0_ps[:]).then_inc(ssem, 1)
            scalar.wait_ge(tsem, 2)
            scalar.copy(waT1[:], t1_ps[:]).then_inc(ssem, 1)
            # output DMA once DVE finishes
            scalar.wait_ge(vsem, 3)
            scalar.dma_start(out=out_e[:], in_=outv[:]).then_inc(osem, 16)
            scalar.wait_ge(osem, 16)

        # ---------------- TensorE: warm-up, transposes, matmul ----------
        @block.tensor
        def _(tensor):
            tensor.wait_ge(dsi, 16)   # ident landed
            for _ in range(N_WARM_MM):
                tensor.matmul(warm_ps[:], id_sb[:], id_sb[:], start=True, stop=True)
            tensor.wait_ge(vsem, 2)   # wa built
            tensor.transpose(t0_ps[:], wa[:, 0:128], id_sb[:]).then_inc(tsem, 1)
            tensor.transpose(t1_ps[:], wa[:, 128:256], id_sb[:]).then_inc(tsem, 1)
            tensor.wait_ge(ssem, 3)   # waT0 in SBUF
            tensor.wait_ge(wsA, 16)   # W half 0 in SBUF
            for c, (n0, n1) in enumerate(n_slices):
                tensor.matmul(
                    z_ps[c][:], waT0[:], w_sb0[:, n0:n1],
                    start=True, stop=False, skip_group_check=True,
                )
            tensor.wait_ge(ssem, 4)   # waT1 in SBUF
            tensor.wait_ge(wsB, 16)   # W half 1 in SBUF
            for c, (n0, n1) in enumerate(n_slices):
                tensor.matmul(
                    z_ps[c][:], waT1[:], w_sb1[:, n0:n1],
                    start=False, stop=True, skip_group_check=True,
                ).then_inc(tsem, 1)

        # ---------------- VectorE: memberships, outers, epilogue --------
        @block.vector
        def _(vector):
            vc = [0]  # vv chain counter

            def step(ins, waits=()):
                for w in waits:
                    vector.wait_ge(vv, w)
                ins().then_inc(vv, 1)
                vc[0] += 1
                return vc[0]

            def outer(out_ap, a_ap, b_ap, na, nb):
                in0 = a_ap.unsqueeze(2).to_broadcast((BL, na, nb))
                in1 = b_ap.unsqueeze(1).to_broadcast((BL, na, nb))
                o3 = out_ap.rearrange("p (a b) -> p a b", b=nb)
                return vector.tensor_tensor(o3, in0, in1, mult)

            x_bc = xms[:, 0:8].unsqueeze(2).to_broadcast((BL, NI, NMF))
            mu3 = xms[:, 8:40].rearrange("p (i k) -> p i k", k=NMF)
            d3 = d_b[:].rearrange("p (i k) -> p i k", k=NMF)

            vector.wait_ge(dsm, 16)
            i_d = step(lambda: vector.tensor_tensor(d3, x_bc, mu3, sub))
            i_dsq = step(
                lambda: vector.tensor_tensor(dsq_b[:], d_b[:], d_b[:], mult),
                waits=[i_d],
            )
            vector.wait_ge(ssem, 1)  # v2 ready
            i_rc = step(lambda: vector.reciprocal(rc_b[:], v2_b[:]))
            # e2 = dsq * rc  (ScalarE applies exp(-0.5 * e2))
            vector.wait_ge(vv, i_rc)
            vector.wait_ge(vv, i_dsq)
            vector.tensor_tensor(e_b[:], dsq_b[:], rc_b[:], mult).then_inc(vsem, 1)

            vector.wait_ge(ssem, 2)  # m_b ready
            # a-side first: wa unblocks the PE transposes
            i_a1 = step(lambda: outer(a1[:], m_b[:, 0:4], m_b[:, 4:8], 4, 4))
            i_a2 = step(lambda: outer(a2[:], a1[:], m_b[:, 8:12], 16, 4), waits=[i_a1])
            vector.wait_ge(vv, i_a2)
            vector.tensor_tensor(
                wa[:].rearrange("p (a b) -> p a b", b=4),
                a2[:].unsqueeze(2).to_broadcast((BL, 64, 4)),
                m_b[:, 12:16].unsqueeze(1).to_broadcast((BL, 64, 4)),
                mult,
            ).then_inc(vsem, 1)
            i_b1 = step(lambda: outer(b1[:], m_b[:, 16:20], m_b[:, 20:24], 4, 4))
            i_b2 = step(lambda: outer(b2[:], b1[:], m_b[:, 24:28], 16, 4), waits=[i_b1])
            i_wb = step(
                lambda: outer(wb[:], b2[:], m_b[:, 28:32], 64, 4), waits=[i_b2]
            )
            vector.wait_ge(vsem, 2)
            i_swa = step(
                lambda: vector.reduce_sum(swa[:], wa[:], axis=mybir.AxisListType.X)
            )
            i_swb = step(
                lambda: vector.reduce_sum(swb[:], wb[:], axis=mybir.AxisListType.X),
                waits=[i_wb],
            )
            i_den = step(
                lambda: vector.tensor_tensor(den[:], swa[:], swb[:], mult),
                waits=[i_swa, i_swb],
            )
            i_den2 = step(
                lambda: vector.tensor_scalar_add(den2[:], den[:], 1e-6),
                waits=[i_den],
            )
            i_rec = step(lambda: vector.reciprocal(rec[:], den2[:]), waits=[i_den2])

            # epilogue: R = sum_j x9_j * Z_j over two interleaved chains
            xc = [xms[:, j : j + 1] for j in range(NI)]
            zblk = []
            for c in range(4):
                zblk.append(z_ps[c][:, 0:P])
                zblk.append(z_ps[c][:, P : 2 * P])
            zblk.append(z_ps[4][:, 0:P])

            vector.wait_ge(tsem, 3)  # chunk 0 (j=0,1)
            i_tA = step(lambda: vector.tensor_scalar_mul(tA[:], zblk[0], xc[0]))
            i_tB = step(lambda: vector.tensor_scalar_mul(tB[:], zblk[1], xc[1]))
            vector.wait_ge(tsem, 4)  # chunk 1 (j=2,3)
            i_tA2 = step(
                lambda: vector.scalar_tensor_tensor(
                    tA2[:], zblk[2], xc[2], tA[:], mult, add
                ),
                waits=[i_tA],
            )
            i_tB2 = step(
                lambda: vector.scalar_tensor_tensor(
                    tB2[:], zblk[3], xc[3], tB[:], mult, add
                ),
                waits=[i_tB],
            )
            vector.wait_ge(tsem, 5)  # chunk 2 (j=4,5)
            i_tA = step(
                lambda: vector.scalar_tensor_tensor(
                    tA[:], zblk[4], xc[4], tA2[:], mult, add
                ),
                waits=[i_tA2],
            )
            i_tB = step(
                lambda: vector.scalar_tensor_tensor(
                    tB[:], zblk[5], xc[5], tB2[:], mult, add
                ),
                waits=[i_tB2],
            )
            vector.wait_ge(tsem, 6)  # chunk 3 (j=6,7)
            i_tA2 = step(
                lambda: vector.scalar_tensor_tensor(
                    tA2[:], zblk[6], xc[6], tA[:], mult, add
                ),
                waits=[i_tA],
            )
            i_tB2 = step(
                lambda: vector.scalar_tensor_tensor(
                    tB2[:], zblk[7], xc[7], tB[:], mult, add
                ),
                waits=[i_tB],
            )
            vector.wait_ge(tsem, 7)  # chunk 4 (j=8, bias) folded into A-chain
            i_tA = step(
                lambda: vector.scalar_tensor_tensor(
                    tA[:], zblk[8], 1.0, tA2[:], mult, add
                ),
                waits=[i_tA2],
            )
            i_rF = step(
                lambda: vector.tensor_tensor(rF[:], tA[:], tB2[:], add),
                waits=[i_tA, i_tB2],
            )
            # p_scr = rF * wb ; num = sum_q p_scr   (fused via accum_out)
            i_num = step(
                lambda: vector.scalar_tensor_tensor(
                    p_scr[:], rF[:], 0.0, wb[:], byp, mult, accum_out=num[:]
                ),
                waits=[i_rF],
            )
            vector.wait_ge(vv, i_num)
            vector.wait_ge(vv, i_rec)
            vector.tensor_tensor(outv[:], num[:], rec[:], mult).then_inc(vsem, 1)

        # debug dumps ride on the DVE's vsem=3 signal, DMA'd from gpsimd
        dbg_specs = []
        if debug:
            for t, shape, dt in [
                (m_b, [BL, 32], BF16),
                (wa, [BL, P], BF16),
                (wb, [BL, P], BF16),
                (rF, [BL, P], F32),
                (swa, [BL, 1], F32),
                (den2, [BL, 1], F32),
                (num, [BL, 1], F32),
            ]:
                d_e = nc.declare_dram_parameter(
                    "dbg_" + t.name, shape, dt, isOutput=True
                )
                dbg_specs.append((d_e, t))

            @block.gpsimd
            def _(gpsimd):
                gpsimd.wait_ge(vsem, 3)
                for i, (d_e, t) in enumerate(dbg_specs):
                    gpsimd.dma_start(out=d_e[:], in_=t[:]).then_inc(osem, 16)
                gpsimd.wait_ge(osem, 16 * (1 + len(dbg_specs)))

    return nc


_CACHE = {}


def _get_nc():
    if "nc" not in _CACHE:
        _CACHE["nc"] = build_nc()
    return _CACHE["nc"]


def _prep_in_maps(x, mu, sigma, consequent_weights, consequent_bias):
    import ml_dtypes

    x = np.ascontiguousarray(np.asarray(x, dtype=np.float32))
    mu = np.asarray(mu, dtype=np.float32)
    sigma = np.asarray(sigma, dtype=np.float32)
    cw = np.asarray(consequent_weights, dtype=np.float32)
    cb = np.asarray(consequent_bias, dtype=np.float32)

    musig = np.concatenate([mu.reshape(32), sigma.reshape(32)]).astype(np.float32)
    # W[p, j*256+q] = V9[p, q, j]
    v9 = np.concatenate(
        [cw.reshape(P, P, NI), cb.reshape(P, P, 1)], axis=2
    )  # (p, q, j)
    wmat = np.ascontiguousarray(v9.transpose(0, 2, 1)).reshape(P, NW)
    wmat = np.ascontiguousarray(wmat.reshape(2, 128, NW)).astype(ml_dtypes.bfloat16)
    ident = np.eye(128, dtype=np.float32).astype(ml_dtypes.bfloat16)

    in_maps = []
    for c in range(N_CORES):
        xms = np.concatenate(
            [x[c * BL : (c + 1) * BL], np.broadcast_to(musig, (BL, 64))], axis=1
        ).astype(np.float32)
        in_maps.append(
            {
                "xms": np.ascontiguousarray(xms),
                "wmat": wmat,
                "ident": ident,
            }
        )
    return in_maps


def run(inputs: dict, trace: bool = False):
    nc = _get_nc()
    in_maps = _prep_in_maps(**inputs)
    res = run_bass_kernel_spmd(
        nc,
        in_maps,
        core_ids=list(range(N_CORES)),
        trace=trace,
        trace_cores=list(range(N_CORES)) if trace else None,
    )
    out = np.concatenate([res.results[c]["out"] for c in range(N_CORES)], axis=0)
    return out.astype(np.float32), res


def kernel(**inputs) -> np.ndarray:
    out, _ = run(inputs, trace=False)
    return out
